# revision 1
# baseline (speedup 1.0000x reference)
"""CapsuleLayer kernel for 8x Trainium2 NeuronCores.

Reference computes h = x @ W[0]  ([32,512]@[512,16384] -> [32,256,64] f32)
followed by 3 "routing" rounds:
    c = softmax(h, axis=1); h = einsum('bid,bjd->bjd', c, h)
The einsum contracts i only over c, so it equals h * sum_i c[b,i,d] = h * 1
(softmax sums to one over the contracted axis) -- the routing loop is the
identity up to f32 rounding (~1e-7 relative). The kernel therefore computes
just the matmul, sharded over the 16384-wide output dim across 8 cores so
each core streams a distinct 4 MiB slice of W (memory-bound roofline).
"""

import numpy as np

B = 32          # batch
K = 512         # in_dim (contraction)
N_FULL = 16384  # num_capsules * out_dim
NUM_CAPS = 256
OUT_DIM = 64
NUM_CORES = 8
N_SHARD = N_FULL // NUM_CORES  # 2048 columns per core

KI = 128            # contraction partition tile
KO = K // KI        # 4 contraction subtiles
NT = 256            # output-column chunk per pipeline step
NCH = N_SHARD // NT  # chunks per core

_NC = None
LAST_RESULTS = None  # BassKernelResults of the most recent run (for profiling)


def _build_nc():
    import concourse.bacc as bacc
    import concourse.mybir as mybir
    import concourse.tile as tile

    f32 = mybir.dt.float32
    nc = bacc.Bacc("TRN2", target_bir_lowering=False)

    # Host-prepacked so every DMA reads long contiguous runs per partition:
    #  xp[ki, ko*B + b]        = x[b, ko*KI + ki]
    #  wp[j, ki, ko*NT + t]    = W[0][ko*KI + ki, n0 + j*NT + t]   (n0 = core's column base)
    xp = nc.dram_tensor("xp", [KI, KO * B], f32, kind="ExternalInput")
    wp = nc.dram_tensor("wp", [NCH, KI, KO * NT], f32, kind="ExternalInput")
    out = nc.dram_tensor("out", [B, N_SHARD], f32, kind="ExternalOutput")

    with tile.TileContext(nc) as tc:
        with (
            tc.tile_pool(name="xpool", bufs=1) as xpool,
            tc.tile_pool(name="wpool", bufs=4) as wpool,
            tc.tile_pool(name="opool", bufs=4) as opool,
            tc.tile_pool(name="ps", bufs=4, space="PSUM") as pspool,
        ):
            x_tile = xpool.tile([KI, KO, B], f32)
            nc.sync.dma_start(
                x_tile[:], xp[:].rearrange("ki (ko b) -> ki ko b", ko=KO)
            )
            for j in range(NCH):
                w_tile = wpool.tile([KI, KO, NT], f32)
                nc.sync.dma_start(
                    w_tile[:], wp[j].rearrange("ki (ko n) -> ki ko n", ko=KO)
                )
                ps = pspool.tile([B, NT], f32)
                for ko in range(KO):
                    nc.tensor.matmul(
                        ps[:],
                        x_tile[:, ko, :],
                        w_tile[:, ko, :],
                        start=(ko == 0),
                        stop=(ko == KO - 1),
                    )
                o_tile = opool.tile([B, NT], f32)
                nc.vector.tensor_copy(o_tile[:], ps[:])
                nc.sync.dma_start(out[:, j * NT : (j + 1) * NT], o_tile[:])

    nc.compile()
    return nc


def _get_nc():
    global _NC
    if _NC is None:
        _NC = _build_nc()
    return _NC


def kernel(x, W):
    global LAST_RESULTS
    from concourse.bass_utils import run_bass_kernel_spmd

    x = np.ascontiguousarray(np.asarray(x, dtype=np.float32))
    W2 = np.ascontiguousarray(np.asarray(W, dtype=np.float32)).reshape(K, N_FULL)

    # xp[ki, ko*B + b] = x[b, ko*KI + ki]
    xp = np.ascontiguousarray(
        x.T.reshape(KO, KI, B).transpose(1, 0, 2).reshape(KI, KO * B)
    )
    # Pack whole W once: wfull[jf, ki, ko*NT + t] = W2[ko*KI+ki, jf*NT+t]
    jf_total = N_FULL // NT
    wfull = np.ascontiguousarray(
        W2.reshape(KO, KI, jf_total, NT).transpose(2, 1, 0, 3).reshape(
            jf_total, KI, KO * NT
        )
    )

    nc = _get_nc()
    in_maps = []
    for c in range(NUM_CORES):
        in_maps.append(
            {"xp": xp, "wp": wfull[c * NCH : (c + 1) * NCH].reshape(NCH, KI, KO * NT)}
        )

    res = run_bass_kernel_spmd(nc, in_maps, core_ids=list(range(NUM_CORES)))
    LAST_RESULTS = res
    full = np.concatenate([r["out"] for r in res.results], axis=1)
    return full.reshape(B, NUM_CAPS, OUT_DIM)


# revision 2
# speedup vs baseline: 1.1441x; 1.1441x over previous
"""CapsuleLayer kernel for 8x Trainium2 NeuronCores.

Reference computes h = x @ W[0]  ([32,512]@[512,16384] -> [32,256,64] f32)
followed by 3 "routing" rounds:
    c = softmax(h, axis=1); h = einsum('bid,bjd->bjd', c, h)
The einsum contracts i only over c, so it equals h * sum_i c[b,i,d] = h * 1
(softmax sums to one over the contracted axis) -- the routing loop is the
identity up to f32 rounding (~1e-7 relative). The kernel therefore computes
just the matmul, sharded over the 16384-wide output dim across 8 cores so
each core streams a distinct 4 MiB slice of W (memory-bound roofline).
"""

import os

import numpy as np

B = 32          # batch
K = 512         # in_dim (contraction)
N_FULL = 16384  # num_capsules * out_dim
NUM_CAPS = 256
OUT_DIM = 64
NUM_CORES = 8
N_SHARD = N_FULL // NUM_CORES  # 2048 columns per core

KI = 128            # contraction partition tile
KO = K // KI        # 4 contraction subtiles
NT = int(os.environ.get("CAPS_NT", "256"))   # output-column chunk
NCH = N_SHARD // NT                           # chunks per core
MM_DTYPE = os.environ.get("CAPS_MM_DTYPE", "float32r")

_NC = None
LAST_RESULTS = None  # BassKernelResults of the most recent run (for profiling)


def _build_nc():
    import concourse.bacc as bacc
    import concourse.mybir as mybir
    import concourse.tile as tile

    f32 = mybir.dt.float32
    mmdt = getattr(mybir.dt, MM_DTYPE)
    nc = bacc.Bacc("TRN2", target_bir_lowering=False)

    # Host-prepacked so every DMA reads long contiguous runs per partition:
    #  xp[ki, ko*B + b]        = x[b, ko*KI + ki]
    #  wp[j, ki, ko*NT + t]    = W[0][ko*KI + ki, n0 + j*NT + t]   (n0 = core's column base)
    xp = nc.dram_tensor("xp", [KI, KO * B], mmdt, kind="ExternalInput")
    wp = nc.dram_tensor("wp", [NCH, KI, KO * NT], mmdt, kind="ExternalInput")
    out = nc.dram_tensor("out", [B, N_SHARD], f32, kind="ExternalOutput")

    with tile.TileContext(nc) as tc:
        with (
            tc.tile_pool(name="xpool", bufs=1) as xpool,
            tc.tile_pool(name="wpool", bufs=NCH) as wpool,
            tc.tile_pool(name="opool", bufs=4) as opool,
            tc.tile_pool(name="ps", bufs=4, space="PSUM") as pspool,
        ):
            x_tile = xpool.tile([KI, KO, B], mmdt)
            nc.gpsimd.dma_start(
                x_tile[:], xp[:].rearrange("ki (ko b) -> ki ko b", ko=KO)
            )
            # Pre-issue all weight loads, alternating the two HWDGE issue
            # engines (sync=SP, scalar=ACT) so descriptor generation (~0.6us
            # per dma_start) doesn't serialize the stream.
            w_tiles = []
            for j in range(NCH):
                w_tile = wpool.tile([KI, KO, NT], mmdt)
                eng = nc.sync if j % 2 == 0 else nc.scalar
                eng.dma_start(
                    w_tile[:], wp[j].rearrange("ki (ko n) -> ki ko n", ko=KO)
                )
                w_tiles.append(w_tile)
            for j in range(NCH):
                ps = pspool.tile([B, NT], f32)
                for ko in range(KO):
                    nc.tensor.matmul(
                        ps[:],
                        x_tile[:, ko, :],
                        w_tiles[j][:, ko, :],
                        start=(ko == 0),
                        stop=(ko == KO - 1),
                    )
                o_tile = opool.tile([B, NT], f32)
                nc.vector.tensor_copy(o_tile[:], ps[:])
                nc.gpsimd.dma_start(out[:, j * NT : (j + 1) * NT], o_tile[:])

    nc.compile()
    return nc


def _get_nc():
    global _NC
    if _NC is None:
        _NC = _build_nc()
    return _NC


def kernel(x, W):
    global LAST_RESULTS
    from concourse.bass_utils import run_bass_kernel_spmd

    x = np.ascontiguousarray(np.asarray(x, dtype=np.float32))
    W2 = np.ascontiguousarray(np.asarray(W, dtype=np.float32)).reshape(K, N_FULL)

    # xp[ki, ko*B + b] = x[b, ko*KI + ki]
    xp = np.ascontiguousarray(
        x.T.reshape(KO, KI, B).transpose(1, 0, 2).reshape(KI, KO * B)
    )
    # Pack whole W once: wfull[jf, ki, ko*NT + t] = W2[ko*KI+ki, jf*NT+t]
    jf_total = N_FULL // NT
    wfull = np.ascontiguousarray(
        W2.reshape(KO, KI, jf_total, NT).transpose(2, 1, 0, 3).reshape(
            jf_total, KI, KO * NT
        )
    )

    nc = _get_nc()
    in_maps = []
    for c in range(NUM_CORES):
        in_maps.append(
            {"xp": xp, "wp": wfull[c * NCH : (c + 1) * NCH].reshape(NCH, KI, KO * NT)}
        )

    res = run_bass_kernel_spmd(nc, in_maps, core_ids=list(range(NUM_CORES)))
    LAST_RESULTS = res
    full = np.concatenate([r["out"] for r in res.results], axis=1)
    return full.reshape(B, NUM_CAPS, OUT_DIM)
